# revision 22
# baseline (speedup 1.0000x reference)
# Block-circulant linear kernel for Trainium2 (Bass, raw engine blocks),
# 8-core SPMD.
#
# y[b, 16m+p] = sum_{n,q} blocks[(m-n)%512, p, q] * x[b, 16n+q]
#
# Strategy: shard the output block axis m across 8 cores (64 block-rows each).
# Per core, store a doubled+shifted "BIGQ" layout of blocks in SBUF:
#     BIGQ[(ni,q), u2*16+p] = blocks[(m0 + 8 + u2 - ni) % 512, p, q]
# so that EVERY 128x128 weight tile of the implied 8192x8192 circulant matrix
# is a contiguous 128-column slice of BIGQ (the circulant gather becomes pure
# addressing). All (m_tile t, n_chunk c) pairs with the same diagonal offset
# d = t - c share one stationary tile, so the whole per-core compute is 71
# accumulating matmuls into a single PSUM bank [128 mp, 8 t x 32 b].
#
# Raw Bass engine blocks with per-DMA-chunk semaphores (a chunk's sem >= 16
# means all 16 SDMA engines finished it -- a single cumulative sem is NOT
# sound because engines complete chunks at independent paces). Each dma_start
# costs ~650ns of issue time on its HWDGE sequencer and transfers below
# ~128KB waste SDMA efficiency, so the streams are split into ~15 chunks,
# assigned greedily across the SP and ACT rings in consumption order
# (matmul i consumes bigq cols [128i,128i+128) and xt cols [0,32(i+1))).
# fp16 warm-up matmuls bridge the preamble-to-first-chunk gap so the PE and
# HAM clock ramp overlap the DMA latency instead of the real stream.
import numpy as np

B = 32
NB = 512          # number of 16x16 blocks
NCORES = 8
MBLK = NB // NCORES   # 64 output block-rows per core
ND = 71               # diagonal offsets d in [-63, 7]
BQW = ND * 128        # 9088 bigq cols actually consumed

NWARM = 12            # fp16 N=512 warm-up matmuls (~213ns full / 426ns gated)
LOOKAHEAD = 0         # extra chunk-windows of pace gating for the PE (0 =
                      # gate only on the chunks a matmul actually reads)

# bigq chunk sizes in units of 128 cols (32KB fp16); bigq unit j is first
# consumed by matmul j. Last chunk kept small so the PE tail after the final
# semaphore is short.
BQ_CHUNKS = [2, 2, 4, 8, 8, 8, 8, 8, 8, 8, 6, 1]
assert sum(BQ_CHUNKS) == ND
# xt chunk sizes in units of 128 cols; xt unit k is first consumed by
# matmul 4k (matmul i needs units 0..i//4).
XT_CHUNKS = [4, 12]
assert sum(XT_CHUNKS) == 16

_cached = {}
_last_results = None  # BassKernelResults of the most recent run (for profiling)


def _chunk_of(chunks, unit):
    cum = 0
    for c, sz in enumerate(chunks):
        cum += sz
        if cum > unit:
            return c
    raise AssertionError


def _build_program():
    import concourse.bacc as bacc
    import concourse.mybir as mybir
    from contextlib import ExitStack

    f16 = mybir.dt.float16
    f32 = mybir.dt.float32

    # Bacc (not plain Bass): its compile() pipeline splits multi-wait
    # instructions into EventSemaphore preludes (HW allows 1 wait/inst).
    nc = bacc.Bacc("TRN2", target_bir_lowering=False, debug=False, num_devices=NCORES)
    xt_d = nc.declare_dram_parameter("xt", [128, 2048], f16, isOutput=False)
    bq_d = nc.declare_dram_parameter("bigq", [128, BQW], f16, isOutput=False)
    # fp16 output (upcast on host): halves the output DMA and doubles the
    # DVE copy rate; adds ~3e-4 rms on top of the fp16 input quantization.
    out_d = nc.declare_dram_parameter("out", [128, 256], f16, isOutput=True)

    nbq = len(BQ_CHUNKS)
    nxt = len(XT_CHUNKS)

    # chunk col ranges and first-needed matmul index
    bq_lims, bq_need = [], []
    cum = 0
    for sz in BQ_CHUNKS:
        bq_lims.append((128 * cum, 128 * (cum + sz)))
        bq_need.append(cum)
        cum += sz
    xt_lims, xt_need = [], []
    cum = 0
    for sz in XT_CHUNKS:
        xt_lims.append((128 * cum, 128 * (cum + sz)))
        xt_need.append(4 * cum)
        cum += sz

    # greedy byte-balanced ring assignment in global consumption order; each
    # ring's own issue order stays need-sorted (HWDGE rings are FIFO).
    todo = sorted(
        [(bq_need[c], "bq", c, BQ_CHUNKS[c]) for c in range(nbq)]
        + [(xt_need[c], "xt", c, XT_CHUNKS[c]) for c in range(nxt)]
    )
    ring_s, ring_a = [], []
    bytes_s = bytes_a = 0
    for need, kind, c, units in todo:
        if bytes_s <= bytes_a:
            ring_s.append((kind, c))
            bytes_s += units
        else:
            ring_a.append((kind, c))
            bytes_a += units

    with ExitStack() as ctx:
        xt = ctx.enter_context(nc.sbuf_tensor("xt_sb", [128, 2048], f16))
        bq = ctx.enter_context(nc.sbuf_tensor("bq_sb", [128, BQW], f16))
        out_sb = ctx.enter_context(nc.sbuf_tensor("out_sb", [128, 256], f16))
        warm_sb = ctx.enter_context(nc.sbuf_tensor("warm_sb", [128, 512], f16))
        acc_t = ctx.enter_context(nc.psum_tensor("acc_ps", [128, 512], f32))
        warm_t = ctx.enter_context(nc.psum_tensor("warm_ps", [128, 512], f32))
        sem_bq = [ctx.enter_context(nc.semaphore(f"sem_bq{c}")) for c in range(nbq)]
        sem_xt = [ctx.enter_context(nc.semaphore(f"sem_xt{c}")) for c in range(nxt)]
        sem_mm = ctx.enter_context(nc.semaphore("sem_mm"))
        sem_cp = ctx.enter_context(nc.semaphore("sem_cp"))
        sem_out = ctx.enter_context(nc.semaphore("sem_out"))

        acc = acc_t[:, 0:256]

        # contiguous ranges for 1-instruction semaphore restore
        in_sems = [s.num for s in sem_bq] + [s.num for s in sem_xt] + [sem_mm.num]
        assert max(in_sems) - min(in_sems) + 1 == len(in_sems), in_sems
        in_rng = range(min(in_sems), max(in_sems) + 1)
        tail_sems = [sem_cp.num, sem_out.num]
        tail_rng = range(min(tail_sems), max(tail_sems) + 1)

        def issue(eng, kind, c):
            if kind == "bq":
                lo, hi = bq_lims[c]
                eng.dma_start(bq[:, lo:hi], bq_d[:, lo:hi]).then_inc(sem_bq[c], 16)
            else:
                lo, hi = xt_lims[c]
                eng.dma_start(xt[:, lo:hi], xt_d[:, lo:hi]).then_inc(sem_xt[c], 16)

        # --- straight-line emission into the entry block (no Block bodies:
        # a branch into a fresh basic block costs ~0.5us of iram fetch before
        # the first DMA can issue). Only per-engine order matters.

        # SP ring
        for kind, c in ring_s:
            issue(nc.sync, kind, c)
        # ACT ring
        for kind, c in ring_a:
            issue(nc.scalar, kind, c)

        # DVE: warm-operand clear, then the PSUM->SBUF cast in two halves so
        # each output DMA can launch as soon as its half lands.
        nc.vector.memset(warm_sb[:], 0.0)
        nc.vector.wait_ge(sem_mm, 1)
        nc.vector.tensor_copy(out_sb[:, 128:256], acc[:, 128:256]).then_inc(sem_cp, 1)
        nc.vector.tensor_copy(out_sb[:, 0:128], acc[:, 0:128]).then_inc(sem_cp, 1)

        # PE: warm-up from preamble end until the first chunks arrive
        # (overlaps the HAM clock ramp with the DMA launch latency), then the
        # 71-diagonal stream.
        for wi in range(NWARM):
            nc.tensor.matmul(
                warm_t[:], warm_sb[:, 0:128], warm_sb[:],
                start=(wi == 0), stop=(wi == NWARM - 1),
            )
        pos_of = {(kind, c): p for p, (_n, kind, c, _u) in enumerate(todo)}
        sem_of = {}
        for c in range(nbq):
            sem_of[("bq", c)] = sem_bq[c]
        for c in range(nxt):
            sem_of[("xt", c)] = sem_xt[c]
        waited = set()

        def gate(kind, c):
            if (kind, c) not in waited:
                nc.tensor.wait_ge(sem_of[(kind, c)], 16)
                waited.add((kind, c))

        for i in range(ND):
            d = i - 63
            t_lo = max(0, d)
            t_hi = min(7, 63 + d)
            nt = t_hi - t_lo + 1
            tp_lo = 7 - t_hi           # flipped psum tile index
            cp_lo = 63 + d - t_hi      # reversed xt chunk index
            cb = _chunk_of(BQ_CHUNKS, i)
            cx = _chunk_of(XT_CHUNKS, min(15, i // 4))
            # correctness gates
            gate("bq", cb)
            gate("xt", cx)
            # pace gate: stay ~LOOKAHEAD chunk-windows behind the completion
            # line so per-chunk receipt latency never stalls the PE mid-run.
            pp = max(pos_of[("bq", cb)], pos_of[("xt", cx)]) + 2 * LOOKAHEAD
            if pp < len(todo):
                _n, pk, pc, _u = todo[pp]
                gate(pk, pc)
            mm = nc.tensor.matmul(
                acc[:, 32 * tp_lo: 32 * (tp_lo + nt)],
                bq[:, 128 * i: 128 * (i + 1)],
                xt[:, 32 * cp_lo: 32 * (cp_lo + nt)],
                start=(i == 0),   # clears the whole PSUM bank
                stop=(i == ND - 1),
                skip_group_check=True,
            )
            if i == ND - 1:
                mm.then_inc(sem_mm, 1)

        # output DMAs: one half per HWDGE ring, launched as its cast lands
        nc.sync.wait_ge(sem_cp, 1)
        nc.sync.dma_start(out_d[:, 128:256], out_sb[:, 128:256]).then_inc(sem_out, 16)
        nc.scalar.wait_ge(sem_cp, 2)
        nc.scalar.dma_start(out_d[:, 0:128], out_sb[:, 0:128]).then_inc(sem_out, 16)
        # semaphore restore: input-side sems are final once the first cast
        # has happened; the out sems once both output DMAs completed. No
        # explicit retire barrier -- walrus emits its own NEFF-end barrier.
        nc.scalar.sem_clear(in_rng)
        nc.sync.wait_ge(sem_out, 32)
        nc.sync.sem_clear(tail_rng)

    nc.compile()
    return nc


def _get_program():
    if "prog" not in _cached:
        _cached["prog"] = _build_program()
    return _cached["prog"]


def _prep_inputs(x, blocks):
    """Host-side layout prep (pure numpy reshuffles of the small inputs)."""
    x = np.ascontiguousarray(np.asarray(x), dtype=np.float32)
    blocks = np.ascontiguousarray(np.asarray(blocks), dtype=np.float32)
    # xt[(ni*16+q), c*32+b] = x[b, 128c + 16ni + q], then reverse c (c'=63-c)
    xt = x.T.reshape(64, 128, 32).transpose(1, 0, 2)[:, ::-1, :].reshape(128, 2048)
    xt = np.ascontiguousarray(xt.astype(np.float16))
    u2 = np.arange(8, 8 + BQW // 16)   # u window actually consumed
    ni = np.arange(8)
    in_maps = []
    for k in range(NCORES):
        m0 = k * MBLK
        idx = (m0 + u2[None, :] - ni[:, None]) % NB       # [8, 568]
        bigq = blocks[idx]                                 # [8, 568, p, q]
        bigq = bigq.transpose(0, 3, 1, 2).reshape(128, BQW)  # [(ni,q), (u,p)]
        in_maps.append({"xt": xt, "bigq": np.ascontiguousarray(bigq.astype(np.float16))})
    return in_maps


def _assemble(results):
    y = np.empty((B, NB * 16), dtype=np.float32)
    for k in range(NCORES):
        # [128 (mi,p), 256 (t',b)] fp16, t = 7-t'; upcast on host
        o = np.asarray(results[k]["out"]).astype(np.float32)
        y[:, 1024 * k: 1024 * (k + 1)] = (
            o.reshape(128, 8, 32)[:, ::-1, :].transpose(2, 1, 0).reshape(32, 1024)
        )
    return y


def kernel(x, blocks):
    global _last_results
    from concourse.bass_utils import run_bass_kernel_spmd

    nc = _get_program()
    in_maps = _prep_inputs(x, blocks)
    res = run_bass_kernel_spmd(nc, in_maps, list(range(NCORES)))
    _last_results = res
    return _assemble(res.results)


# revision 25
# speedup vs baseline: 1.0175x; 1.0175x over previous
# Block-circulant linear kernel for Trainium2 (Bass, raw engine blocks),
# 8-core SPMD.
#
# y[b, 16m+p] = sum_{n,q} blocks[(m-n)%512, p, q] * x[b, 16n+q]
#
# Strategy: shard the output block axis m across 8 cores (64 block-rows each).
# Per core, store a doubled+shifted "BIGQ" layout of blocks in SBUF:
#     BIGQ[(ni,q), u2*16+p] = blocks[(m0 + 8 + u2 - ni) % 512, p, q]
# so that EVERY 128x128 weight tile of the implied 8192x8192 circulant matrix
# is a contiguous 128-column slice of BIGQ (the circulant gather becomes pure
# addressing). All (m_tile t, n_chunk c) pairs with the same diagonal offset
# d = t - c share one stationary tile, so the whole per-core compute is 71
# accumulating matmuls into a single PSUM bank [128 mp, 8 t x 32 b].
#
# Raw Bass engine blocks with per-DMA-chunk semaphores (a chunk's sem >= 16
# means all 16 SDMA engines finished it -- a single cumulative sem is NOT
# sound because engines complete chunks at independent paces). Each dma_start
# costs ~650ns of issue time on its HWDGE sequencer and transfers below
# ~128KB waste SDMA efficiency, so the streams are split into ~15 chunks,
# assigned greedily across the SP and ACT rings in consumption order
# (matmul i consumes bigq cols [128i,128i+128) and xt cols [0,32(i+1))).
# fp16 warm-up matmuls bridge the preamble-to-first-chunk gap so the PE and
# HAM clock ramp overlap the DMA latency instead of the real stream.
import numpy as np

B = 32
NB = 512          # number of 16x16 blocks
NCORES = 8
MBLK = NB // NCORES   # 64 output block-rows per core
ND = 71               # diagonal offsets d in [-63, 7]
BQW = ND * 128        # 9088 bigq cols actually consumed

NWARM = 14            # fp16 N=512 warm-up matmuls (~213ns full / 426ns gated)
LOOKAHEAD = 0         # extra chunk-windows of pace gating for the PE (0 =
                      # gate only on the chunks a matmul actually reads)

# bigq chunk sizes in units of 128 cols (32KB fp16); bigq unit j is first
# consumed by matmul j. Last chunk kept small so the PE tail after the final
# semaphore is short.
BQ_CHUNKS = [2, 2, 4, 8, 8, 8, 8, 8, 8, 8, 6, 1]
assert sum(BQ_CHUNKS) == ND
# xt chunk sizes in units of 128 cols; xt unit k is first consumed by
# matmul 4k (matmul i needs units 0..i//4).
XT_CHUNKS = [4, 12]
assert sum(XT_CHUNKS) == 16

_cached = {}
_last_results = None  # BassKernelResults of the most recent run (for profiling)


def _chunk_of(chunks, unit):
    cum = 0
    for c, sz in enumerate(chunks):
        cum += sz
        if cum > unit:
            return c
    raise AssertionError


def _build_program():
    import concourse.bacc as bacc
    import concourse.mybir as mybir
    from contextlib import ExitStack

    f16 = mybir.dt.float16
    f32 = mybir.dt.float32

    # Bacc (not plain Bass): its compile() pipeline splits multi-wait
    # instructions into EventSemaphore preludes (HW allows 1 wait/inst).
    nc = bacc.Bacc("TRN2", target_bir_lowering=False, debug=False, num_devices=NCORES)
    xt_d = nc.declare_dram_parameter("xt", [128, 2048], f16, isOutput=False)
    bq_d = nc.declare_dram_parameter("bigq", [128, BQW], f16, isOutput=False)
    # fp16 output (upcast on host): halves the output DMA and doubles the
    # DVE copy rate; adds ~3e-4 rms on top of the fp16 input quantization.
    out_d = nc.declare_dram_parameter("out", [128, 256], f16, isOutput=True)

    nbq = len(BQ_CHUNKS)
    nxt = len(XT_CHUNKS)

    # chunk col ranges and first-needed matmul index
    bq_lims, bq_need = [], []
    cum = 0
    for sz in BQ_CHUNKS:
        bq_lims.append((128 * cum, 128 * (cum + sz)))
        bq_need.append(cum)
        cum += sz
    xt_lims, xt_need = [], []
    cum = 0
    for sz in XT_CHUNKS:
        xt_lims.append((128 * cum, 128 * (cum + sz)))
        xt_need.append(4 * cum)
        cum += sz

    # greedy byte-balanced ring assignment in global consumption order; each
    # ring's own issue order stays need-sorted (HWDGE rings are FIFO).
    todo = sorted(
        [(bq_need[c], "bq", c, BQ_CHUNKS[c]) for c in range(nbq)]
        + [(xt_need[c], "xt", c, XT_CHUNKS[c]) for c in range(nxt)]
    )
    ring_s, ring_a = [], []
    bytes_s = bytes_a = 0
    for need, kind, c, units in todo:
        if bytes_s <= bytes_a:
            ring_s.append((kind, c))
            bytes_s += units
        else:
            ring_a.append((kind, c))
            bytes_a += units

    with ExitStack() as ctx:
        xt = ctx.enter_context(nc.sbuf_tensor("xt_sb", [128, 2048], f16))
        bq = ctx.enter_context(nc.sbuf_tensor("bq_sb", [128, BQW], f16))
        out_sb = ctx.enter_context(nc.sbuf_tensor("out_sb", [128, 256], f16))
        warm_sb = ctx.enter_context(nc.sbuf_tensor("warm_sb", [128, 512], f16))
        acc_t = ctx.enter_context(nc.psum_tensor("acc_ps", [128, 512], f32))
        warm_t = ctx.enter_context(nc.psum_tensor("warm_ps", [128, 512], f32))
        sem_bq = [ctx.enter_context(nc.semaphore(f"sem_bq{c}")) for c in range(nbq)]
        sem_xt = [ctx.enter_context(nc.semaphore(f"sem_xt{c}")) for c in range(nxt)]
        sem_mm = ctx.enter_context(nc.semaphore("sem_mm"))
        sem_cp = ctx.enter_context(nc.semaphore("sem_cp"))
        sem_out = ctx.enter_context(nc.semaphore("sem_out"))

        acc = acc_t[:, 0:256]

        # contiguous range for 1-instruction semaphore restore
        all_sems = (
            [s.num for s in sem_bq] + [s.num for s in sem_xt]
            + [sem_mm.num, sem_cp.num, sem_out.num]
        )
        assert max(all_sems) - min(all_sems) + 1 == len(all_sems), all_sems
        all_rng = range(min(all_sems), max(all_sems) + 1)

        def issue(eng, kind, c):
            if kind == "bq":
                lo, hi = bq_lims[c]
                eng.dma_start(bq[:, lo:hi], bq_d[:, lo:hi]).then_inc(sem_bq[c], 16)
            else:
                lo, hi = xt_lims[c]
                eng.dma_start(xt[:, lo:hi], xt_d[:, lo:hi]).then_inc(sem_xt[c], 16)

        # --- straight-line emission into the entry block (no Block bodies:
        # a branch into a fresh basic block costs ~0.5us of iram fetch before
        # the first DMA can issue). Only per-engine order matters.

        # SP ring
        for kind, c in ring_s:
            issue(nc.sync, kind, c)
        # ACT ring
        for kind, c in ring_a:
            issue(nc.scalar, kind, c)

        # DVE: warm-operand clear, then the PSUM->SBUF cast in two halves so
        # each output DMA can launch as soon as its half lands.
        nc.vector.memset(warm_sb[:], 0.0)
        nc.vector.wait_ge(sem_mm, 1)
        nc.vector.tensor_copy(out_sb[:, 128:256], acc[:, 128:256]).then_inc(sem_cp, 1)
        nc.vector.tensor_copy(out_sb[:, 0:128], acc[:, 0:128]).then_inc(sem_cp, 1)

        # PE: warm-up from preamble end until the first chunks arrive
        # (overlaps the HAM clock ramp with the DMA launch latency), then the
        # 71-diagonal stream.
        for wi in range(NWARM):
            nc.tensor.matmul(
                warm_t[:], warm_sb[:, 0:128], warm_sb[:],
                start=(wi == 0), stop=(wi == NWARM - 1),
            )
        pos_of = {(kind, c): p for p, (_n, kind, c, _u) in enumerate(todo)}
        sem_of = {}
        for c in range(nbq):
            sem_of[("bq", c)] = sem_bq[c]
        for c in range(nxt):
            sem_of[("xt", c)] = sem_xt[c]
        waited = set()

        def gate(kind, c):
            if (kind, c) not in waited:
                nc.tensor.wait_ge(sem_of[(kind, c)], 16)
                waited.add((kind, c))

        for i in range(ND):
            d = i - 63
            t_lo = max(0, d)
            t_hi = min(7, 63 + d)
            nt = t_hi - t_lo + 1
            tp_lo = 7 - t_hi           # flipped psum tile index
            cp_lo = 63 + d - t_hi      # reversed xt chunk index
            cb = _chunk_of(BQ_CHUNKS, i)
            cx = _chunk_of(XT_CHUNKS, min(15, i // 4))
            # correctness gates
            gate("bq", cb)
            gate("xt", cx)
            # pace gate: stay ~LOOKAHEAD chunk-windows behind the completion
            # line so per-chunk receipt latency never stalls the PE mid-run.
            pp = max(pos_of[("bq", cb)], pos_of[("xt", cx)]) + 2 * LOOKAHEAD
            if pp < len(todo):
                _n, pk, pc, _u = todo[pp]
                gate(pk, pc)
            mm = nc.tensor.matmul(
                acc[:, 32 * tp_lo: 32 * (tp_lo + nt)],
                bq[:, 128 * i: 128 * (i + 1)],
                xt[:, 32 * cp_lo: 32 * (cp_lo + nt)],
                start=(i == 0),   # clears the whole PSUM bank
                stop=(i == ND - 1),
                skip_group_check=True,
            )
            if i == ND - 1:
                mm.then_inc(sem_mm, 1)

        # output DMAs: one half per HWDGE ring, launched as its cast lands
        nc.sync.wait_ge(sem_cp, 1)
        nc.sync.dma_start(out_d[:, 128:256], out_sb[:, 128:256]).then_inc(sem_out, 16)
        nc.scalar.wait_ge(sem_cp, 2)
        nc.scalar.dma_start(out_d[:, 0:128], out_sb[:, 0:128]).then_inc(sem_out, 16)
        # completion wait + semaphore restore live on the otherwise-idle
        # GPSIMD engine: the busy engines retire at their last issue and run
        # their (~1-2us) walrus postambles IN PARALLEL with the output DMA
        # flight + HBM write receipt. No explicit retire barrier -- walrus
        # emits its own NEFF-end barrier.
        nc.gpsimd.wait_ge(sem_out, 32)
        nc.gpsimd.sem_clear(all_rng)

    nc.compile()
    return nc


def _get_program():
    if "prog" not in _cached:
        _cached["prog"] = _build_program()
    return _cached["prog"]


def _prep_inputs(x, blocks):
    """Host-side layout prep (pure numpy reshuffles of the small inputs)."""
    x = np.ascontiguousarray(np.asarray(x), dtype=np.float32)
    blocks = np.ascontiguousarray(np.asarray(blocks), dtype=np.float32)
    # xt[(ni*16+q), c*32+b] = x[b, 128c + 16ni + q], then reverse c (c'=63-c)
    xt = x.T.reshape(64, 128, 32).transpose(1, 0, 2)[:, ::-1, :].reshape(128, 2048)
    xt = np.ascontiguousarray(xt.astype(np.float16))
    u2 = np.arange(8, 8 + BQW // 16)   # u window actually consumed
    ni = np.arange(8)
    in_maps = []
    for k in range(NCORES):
        m0 = k * MBLK
        idx = (m0 + u2[None, :] - ni[:, None]) % NB       # [8, 568]
        bigq = blocks[idx]                                 # [8, 568, p, q]
        bigq = bigq.transpose(0, 3, 1, 2).reshape(128, BQW)  # [(ni,q), (u,p)]
        in_maps.append({"xt": xt, "bigq": np.ascontiguousarray(bigq.astype(np.float16))})
    return in_maps


def _assemble(results):
    y = np.empty((B, NB * 16), dtype=np.float32)
    for k in range(NCORES):
        # [128 (mi,p), 256 (t',b)] fp16, t = 7-t'; upcast on host
        o = np.asarray(results[k]["out"]).astype(np.float32)
        y[:, 1024 * k: 1024 * (k + 1)] = (
            o.reshape(128, 8, 32)[:, ::-1, :].transpose(2, 1, 0).reshape(32, 1024)
        )
    return y


def kernel(x, blocks):
    global _last_results
    from concourse.bass_utils import run_bass_kernel_spmd

    nc = _get_program()
    in_maps = _prep_inputs(x, blocks)
    res = run_bass_kernel_spmd(nc, in_maps, list(range(NCORES)))
    _last_results = res
    return _assemble(res.results)
